# revision 1
# baseline (speedup 1.0000x reference)
"""Data-adaptive weight-ensembling MLP (per-sample expert-merged FFN) on 8 trn2 cores.

Math (per sample b):
  c[b,:,:]  = gate(x)[b].reshape(E, L)          (2-layer relu MLP gate)
  W1[b] = bW1 + sum_e c[b,e,0] tvW1[e];  b1[b] = bb1 + sum_e c[b,e,1] tvb1[e]
  W2[b] = bW2 + sum_e c[b,e,2] tvW2[e];  b2[b] = bb2 + sum_e c[b,e,3] tvb2[e]
  out[b] = relu(x[b] @ W1[b].T + b1[b]) @ W2[b].T + b2[b]

Key transforms vs a direct evaluation:
  1. delta-fold: c[b,e,l] = gb2[e,l] + delta[b,e,l] where delta is the
     (small, ~0.07) data-dependent gate output.  The sample-independent
     gb2-weighted expert sum is folded into the base weights on the host:
       bW1' = bW1 + sum_e gb2[e,0] tvW1[e]   (etc. for b1, W2, b2)
     so the device only streams the task-vector bank against the small
     delta coefficients.  This cuts fp8 quantization noise ~3x and is what
     makes fp8 banks fit the 2e-2 error budget.
  2. fp8(e4m3) task-vector banks scaled by S1=64, stationary x*delta*G1=16
     in fp8, base weights bf16 scaled by SC=S1*G1=1024 so every matmul
     lands in one PSUM accumulation at a common scale; un-scaled on the
     psum->sbuf copy (DVE tensor_scalar).  DoubleRow perf mode runs the
     fp8 expert matmuls at 2 K-chunks per instruction (0.5 cycles/row).
     Gate W1 streams in fp8 scaled by SG=128 (only perturbs the small
     delta coefficients).
  3. Merged weights never materialize: for expert e the stationary operand
     is X1T[e][d, b] = x[b, d] * delta[b, e, 0] * G1 and the bank streams
     through PSUM accumulation exactly once.
  4. DMA-lean layouts: biases ride as row 0 of the [1+E, .] tv-bias
     tensors against a [ones; deltaT] stationary; xT+gW2 share one packed
     DMA; the layer-2 bank is split per output half so the first half of
     the output DMAs out while the second half still streams.

Sharding (8 cores): DFF=4096 split into 8 slices of 512.  Core k computes
layer-1 columns in its slice (exact local relu), then contracts layer 2
over the same f-slice producing a partial [B, D].  The 8 partials are
summed on the host (the unshard step) - no device collective.
Task-vector banks are sharded along DFF (~17MB fp8 per core, the DMA
roofline); gate + base weights replicated/sliced.
"""

import contextlib

import numpy as np

B, D, DFF, E, L = 16, 1024, 4096, 16, 4
NCORES = 8
OSL = DFF // NCORES          # 512: per-core DFF slice
KC1 = D // 128               # 8 k-chunks for the d contraction
KC2 = OSL // 128             # 4 k-chunks for the f contraction
S1 = 64.0                    # fp8 scale on the tv banks
G1 = 16.0                    # fp8 scale on the stationary x*delta
SC = S1 * G1                 # resulting PSUM scale
SG = 128.0                   # fp8 scale on gate W1

_cache = {}


def _build(reps: int = 1, collective: bool = False, cfg: str | None = None):
    import concourse.bacc as bacc
    import concourse.bass as bass  # noqa: F401
    import concourse.tile as tile
    import concourse.mybir as mybir
    from concourse.masks import make_identity

    if cfg is None:
        cfg = CFG
    f32 = mybir.dt.float32
    bf16 = mybir.dt.bfloat16
    f8 = mybir.dt.float8e4
    # fp8 stationary + DoubleRow unless the precision-fallback cfg is chosen
    stat_dt = bf16 if cfg == "mixed" else f8
    DR = None if cfg == "mixed" else mybir.MatmulPerfMode.DoubleRow
    PAIR = 1 if DR is None else 2   # k-chunks per expert matmul
    mlt = mybir.AluOpType.mult
    mx = mybir.AluOpType.max
    Copy = mybir.ActivationFunctionType.Copy
    nc = bacc.Bacc("TRN2", target_bir_lowering=False, debug=False,
                   num_devices=NCORES)

    # ---- I/O (per-core data supplied via in_maps) ----
    pk1_h = nc.dram_tensor("pk1", [128, KC1, B + E * L], bf16,
                           kind="ExternalInput")   # xT ++ gW2.T
    gw1_h = nc.dram_tensor("gw1", [128, KC1, D], f8, kind="ExternalInput")
    gb1_h = nc.dram_tensor("gb1r", [16, D], f8, kind="ExternalInput")
    tv1_h = nc.dram_tensor("tv1", [E, 128, KC1, OSL], f8,
                           kind="ExternalInput")
    bw1_h = nc.dram_tensor("bw1", [128, KC1, OSL], bf16, kind="ExternalInput")
    tvb1_h = nc.dram_tensor("tvb1e", [1 + E, OSL], bf16,
                            kind="ExternalInput")  # row0 = bb1'
    tv2_h = nc.dram_tensor("tv2", [E, 2, 128, KC2, 512], f8,
                           kind="ExternalInput")
    bw2_h = nc.dram_tensor("bw2", [128, KC2, D], bf16, kind="ExternalInput")
    tvb2_h = nc.dram_tensor("tvb2e", [1 + E, D], f8,
                            kind="ExternalInput")  # row0 = bb2'
    out_h = nc.dram_tensor("out", [B, D], bf16, kind="ExternalOutput")

    with tile.TileContext(nc) as tc, contextlib.ExitStack() as ctx:
        const = ctx.enter_context(tc.tile_pool(name="const", bufs=1))
        small = ctx.enter_context(tc.tile_pool(name="small", bufs=1))
        gwp = ctx.enter_context(tc.tile_pool(name="gwp", bufs=1))
        bankp1 = ctx.enter_context(tc.tile_pool(name="bankp1", bufs=1))
        bankp2 = ctx.enter_context(tc.tile_pool(name="bankp2", bufs=1))
        pacc = ctx.enter_context(tc.tile_pool(name="pacc", bufs=1,
                                              space="PSUM"))
        psml = ctx.enter_context(tc.tile_pool(name="psml", bufs=2,
                                              space="PSUM"))
        psone = ctx.enter_context(tc.tile_pool(name="psone", bufs=1,
                                               space="PSUM"))

        # constants (once)
        ident16 = const.tile([B, B], f32)
        make_identity(nc, ident16[:])
        identG = const.tile([B, B], bf16)       # G1-scaled identity
        nc.scalar.activation(identG[:], ident16[:], Copy, scale=G1)
        ones16s = const.tile([16, B], bf16)     # 1/16 for gb1 replication
        nc.vector.memset(ones16s[:], 1.0 / 16.0)
        ones16_128 = const.tile([B, 128], bf16)
        nc.vector.memset(ones16_128[:], 1.0)

        for _rep in range(reps):
            sfx = f"_{_rep}"
            # ---- DMA on sync queue: gate + layer-1 + layer-2 streams ----
            pk1 = small.tile([128, KC1, B + E * L], bf16, name="pk1" + sfx,
                             tag="pk1")
            nc.sync.dma_start(out=pk1[:], in_=pk1_h.ap())
            xT = pk1[:, :, 0:B]
            gw2t = pk1[:, :, B:B + E * L]
            gb1r = small.tile([16, D], f8, name="gb1r" + sfx, tag="gb1r")
            nc.sync.dma_start(out=gb1r[:], in_=gb1_h.ap())
            gw1t = gwp.tile([128, KC1, D], f8, name="gw1t" + sfx, tag="gw1t")
            nc.sync.dma_start(out=gw1t[:], in_=gw1_h.ap())
            tvb1t = small.tile([1 + E, OSL], bf16, name="tvb1t" + sfx,
                               tag="tvb1t")
            nc.sync.dma_start(out=tvb1t[:], in_=tvb1_h.ap())
            bw1t = small.tile([128, KC1, OSL], bf16, name="bw1t" + sfx,
                              tag="bw1t")
            nc.sync.dma_start(out=bw1t[:], in_=bw1_h.ap())
            tv1t = []
            for e in range(E):
                t = bankp1.tile([128, KC1, OSL], f8, tag=f"tv1_{e}")
                nc.sync.dma_start(out=t[:], in_=tv1_h.ap()[e])
                tv1t.append(t)
            tvb2t = small.tile([1 + E, D], f8, name="tvb2t" + sfx,
                               tag="tvb2t")
            nc.sync.dma_start(out=tvb2t[:], in_=tvb2_h.ap())
            bw2t = small.tile([128, KC2, D], bf16, name="bw2t" + sfx,
                              tag="bw2t")
            nc.sync.dma_start(out=bw2t[:], in_=bw2_h.ap())
            tv2t = {0: [], 1: []}
            for n in range(2):
                for e in range(E):
                    t = bankp2.tile([128, KC2, 512], f8, tag=f"tv2_{n}_{e}")
                    nc.sync.dma_start(out=t[:], in_=tv2_h.ap()[e][n])
                    tv2t[n].append(t)

            # ---- gate layer 1: g_h = relu(x @ gW1.T + gb1) ----
            g_h = small.tile([B, D], f32, name="g_h" + sfx, tag="g_h")
            for n in range(2):
                gps = pacc.tile([B, 512], f32, tag="gps")
                nc.tensor.matmul(gps[:], ones16s[:],
                                 gb1r[:, n * 512:(n + 1) * 512],
                                 start=True, stop=False)
                for kc in range(KC1):
                    nc.tensor.matmul(gps[:], xT[:, kc, :],
                                     gw1t[:, kc, n * 512:(n + 1) * 512],
                                     start=False, stop=(kc == KC1 - 1))
                nc.vector.tensor_scalar(g_h[:, n * 512:(n + 1) * 512],
                                        gps[:], 1.0 / SG, 0.0, mlt, mx)

            # ---- transpose g_h -> ghT [128, (kc, b)] ----
            ghT = small.tile([128, KC1, B], bf16, name="ghT" + sfx, tag="ghT")
            for kc in range(KC1):
                pt = psml.tile([128, B], f32, tag="ps")
                nc.tensor.transpose(pt[:], g_h[:, kc * 128:(kc + 1) * 128],
                                    ident16[:])
                nc.vector.tensor_copy(ghT[:, kc, :], pt[:])

            # ---- gate layer 2 (NO bias): delta[b, e, l] ----
            cps = psone.tile([B, E * L], f32, tag="cps")
            for kc in range(KC1):
                nc.tensor.matmul(cps[:], ghT[:, kc, :], gw2t[:, kc, :],
                                 start=(kc == 0), stop=(kc == KC1 - 1))
            cod = small.tile([B, E, L], f32, name="cod" + sfx, tag="cod")
            nc.vector.tensor_copy(cod[:],
                                  cps[:].rearrange("b (e l) -> b e l", e=E))

            # ---- [ones; deltaT] stationaries for the bias matmuls ----
            cT = {}
            for l in (1, 3):
                cl = small.tile([B, 1 + E], f32, name=f"cl{l}" + sfx,
                                tag=f"cl{l}")
                nc.vector.memset(cl[:, 0:1], 1.0)
                nc.vector.tensor_copy(cl[:, 1:1 + E], cod[:, :, l])
                ptc = psml.tile([128, B], f32, tag="ps")
                nc.tensor.transpose(ptc[0:1 + E, :], cl[:], ident16[:])
                cTl = small.tile([1 + E, B], bf16, name=f"cT{l}" + sfx,
                                 tag=f"cT{l}")
                nc.vector.tensor_copy(cTl[:], ptc[0:1 + E, :])
                cT[l] = cTl

            # ---- broadcast G1*delta over partitions: cbc[l][p, e, b] ----
            cbc = {}
            for l in (0, 2):
                dgc = small.tile([B, E, B], bf16, name=f"dg{l}" + sfx,
                                 tag=f"dg{l}")
                nc.vector.tensor_mul(
                    dgc[:],
                    identG[:, None, :].broadcast_to([B, E, B]),
                    cod[:, :, l:l + 1].broadcast_to([B, E, B]))
                pbc = psone.tile([128, E * B], f32, tag="pbc")
                nc.tensor.matmul(pbc[:], ones16_128[:],
                                 dgc[:].rearrange("b e c -> b (e c)"),
                                 start=True, stop=True)
                cb = small.tile([128, E, B], bf16, name=f"cb{l}" + sfx,
                                tag=f"cb{l}")
                nc.vector.tensor_copy(
                    cb[:], pbc[:].rearrange("p (e c) -> p e c", e=E))
                cbc[l] = cb

            # ---- x1bank[p, e, kc, b] = xT * G1*delta0 (fp8 stationary) ----
            x1bank = small.tile([128, E, KC1, B], stat_dt, name="x1b" + sfx,
                                tag="x1b")
            nc.vector.tensor_mul(
                x1bank[:],
                xT[:, None, :, :].broadcast_to([128, E, KC1, B]),
                cbc[0][:, :, None, :].broadcast_to([128, E, KC1, B]))

            # ---- layer 1: psum1[b, o] = SC * full local pre-activation ----
            psum1 = pacc.tile([B, OSL], f32, tag="psum1")
            nc.tensor.matmul(psum1[:], cT[1][:], tvb1t[:], start=True,
                             stop=False)
            for kc in range(KC1):
                nc.tensor.matmul(psum1[:], xT[:, kc, :], bw1t[:, kc, :],
                                 start=False, stop=False)
            for e in range(E):
                for p in range(0, KC1, PAIR):
                    nc.tensor.matmul(psum1[:],
                                     x1bank[:, e, p:p + PAIR, :],
                                     tv1t[e][:, p:p + PAIR, :],
                                     start=False,
                                     stop=(e == E - 1 and p == KC1 - PAIR),
                                     perf_mode=DR)

            h1 = small.tile([B, OSL], f32, name="h1" + sfx, tag="h1")
            nc.vector.tensor_scalar(h1[:], psum1[:], 1.0 / SC, 0.0, mlt, mx)

            # ---- transpose h1 -> h1T [128, (fc, b)] ----
            h1T = small.tile([128, KC2, B], bf16, name="h1T" + sfx,
                             tag="h1T")
            for fc in range(KC2):
                pt2 = psml.tile([128, B], f32, tag="ps")
                nc.tensor.transpose(pt2[:], h1[:, fc * 128:(fc + 1) * 128],
                                    ident16[:])
                nc.vector.tensor_copy(h1T[:, fc, :], pt2[:])

            # ---- x2bank[p, e, fc, b] = h1T * G1*delta2 (fp8 stationary) ----
            x2bank = small.tile([128, E, KC2, B], stat_dt, name="x2b" + sfx,
                                tag="x2b")
            nc.vector.tensor_mul(
                x2bank[:],
                h1T[:, None, :, :].broadcast_to([128, E, KC2, B]),
                cbc[2][:, :, None, :].broadcast_to([128, E, KC2, B]))

            # ---- layer 2 by output half; first half DMAs out early ----
            for n in range(2):
                ps = psml.tile([B, 512], f32, tag="psum2")
                nc.tensor.matmul(ps[:], cT[3][:],
                                 tvb2t[:, n * 512:(n + 1) * 512],
                                 start=True, stop=False)
                for fc in range(KC2):
                    nc.tensor.matmul(ps[:], h1T[:, fc, :],
                                     bw2t[:, fc, n * 512:(n + 1) * 512],
                                     start=False, stop=False)
                for e in range(E):
                    for p in range(0, KC2, PAIR):
                        nc.tensor.matmul(
                            ps[:],
                            x2bank[:, e, p:p + PAIR, :],
                            tv2t[n][e][:, p:p + PAIR, :],
                            start=False,
                            stop=(e == E - 1 and p == KC2 - PAIR),
                            perf_mode=DR)
                outp = small.tile([B, 512], bf16, name=f"outp{n}" + sfx,
                                  tag=f"outp{n}")
                nc.vector.tensor_scalar_mul(outp[:], ps[:], 1.0 / SC)
                # act queue so the transfer overlaps the n=1 bank stream
                nc.scalar.dma_start(out=out_h.ap()[:, n * 512:(n + 1) * 512],
                                 in_=outp[:])

    nc.compile()
    return nc


def _prep_inputs(x, gW1, gb1, gW2, gb2, bW1, bb1, bW2, bb2,
                 tvW1, tvb1, tvW2, tvb2, cfg: str | None = None):
    """Build the 8 per-core in_maps (delta-fold + scaling + DMA layouts)."""
    import ml_dtypes

    bf = np.dtype(ml_dtypes.bfloat16)
    f8 = np.dtype(ml_dtypes.float8_e4m3)
    f = np.float32
    x, gW1, gb1, gW2, gb2 = [np.asarray(a, f)
                             for a in (x, gW1, gb1, gW2, gb2)]
    bW1, bb1, bW2, bb2 = [np.asarray(a, f) for a in (bW1, bb1, bW2, bb2)]
    tvW1, tvb1, tvW2, tvb2 = [np.asarray(a, f)
                              for a in (tvW1, tvb1, tvW2, tvb2)]

    # delta-fold: base' = base + sum_e gb2[e,l] * tv[e]
    gb2r = gb2.reshape(E, L)
    bW1p = bW1 + np.tensordot(gb2r[:, 0], tvW1, axes=(0, 0))
    bb1p = bb1 + gb2r[:, 1] @ tvb1
    bW2p = bW2 + np.tensordot(gb2r[:, 2], tvW2, axes=(0, 0))
    bb2p = bb2 + gb2r[:, 3] @ tvb2

    xT = x.T.reshape(KC1, 128, B).transpose(1, 0, 2)
    gw2 = gW2.T.reshape(KC1, 128, E * L).transpose(1, 0, 2)
    pk1 = np.ascontiguousarray(np.concatenate([xT, gw2], axis=2)).astype(bf)
    gw1 = np.ascontiguousarray(
        np.clip(gW1.T * SG, -240.0, 240.0)
        .reshape(KC1, 128, D).transpose(1, 0, 2)).astype(f8)
    gb1r = np.ascontiguousarray(
        np.broadcast_to(np.clip(gb1.reshape(1, D) * SG, -240.0, 240.0),
                        (16, D))).astype(f8)

    tv1s = np.clip(tvW1 * S1, -240.0, 240.0).astype(f8)
    tv2s = np.clip(tvW2 * S1, -240.0, 240.0).astype(f8)

    def _bias_pack(b0, tvb, ncols, dt):
        out = np.empty((1 + E, ncols), f)
        out[0] = b0
        out[1:] = tvb
        return np.clip(out * SC, -240.0, 240.0).astype(dt)

    in_maps = []
    for k in range(NCORES):
        o0 = k * OSL
        tv1 = np.ascontiguousarray(
            tv1s[:, o0:o0 + OSL, :].transpose(0, 2, 1)
            .reshape(E, KC1, 128, OSL).transpose(0, 2, 1, 3))
        bw1 = np.ascontiguousarray(
            (bW1p[o0:o0 + OSL, :].T * SC)
            .reshape(KC1, 128, OSL).transpose(1, 0, 2)).astype(bf)
        tv2 = np.ascontiguousarray(
            tv2s[:, :, o0:o0 + OSL].transpose(0, 2, 1)
            .reshape(E, KC2, 128, 2, 512).transpose(0, 3, 2, 1, 4))
        bw2 = np.ascontiguousarray(
            (bW2p[:, o0:o0 + OSL].T * SC)
            .reshape(KC2, 128, D).transpose(1, 0, 2)).astype(bf)
        zero = k != 0   # layer-2 bias terms only once across partial sums
        in_maps.append(dict(
            pk1=pk1, gw1=gw1, gb1r=gb1r,
            tv1=tv1, bw1=bw1,
            tvb1e=_bias_pack(bb1p[o0:o0 + OSL], tvb1[:, o0:o0 + OSL],
                             OSL, bf),
            tv2=tv2, bw2=bw2,
            tvb2e=np.zeros((1 + E, D), f8) if zero
            else _bias_pack(bb2p, tvb2, D, f8),
        ))
    return in_maps


CFG = "fp8"


def kernel(**inputs):
    from concourse.bass_utils import run_bass_kernel_spmd

    key = ("nc", CFG)
    if key not in _cache:
        _cache[key] = _build(cfg=CFG)
    nc = _cache[key]

    in_maps = _prep_inputs(**{k: np.asarray(v) for k, v in inputs.items()},
                           cfg=CFG)
    res = run_bass_kernel_spmd(nc, in_maps, core_ids=list(range(NCORES)))
    # each core holds a partial sum over its DFF slice: unshard = sum
    out = np.zeros((B, D), np.float32)
    for r in res.results:
        out += np.asarray(r["out"], np.float32)
    return out

